# revision 2
# baseline (speedup 1.0000x reference)
"""SAGAN-style attention block (B=16, C=64, H=W=64) on 8 trn2 NeuronCores.

Data-parallel over batch: 2 samples per core.  Per sample:
    g/phi conv -> 2x2 maxpool           (PE + DVE)
    phi~ = Wt^T @ phi                   (PE)   [folds theta away: scoresT = phi~^T x]
    scoresT[s,t] = phi~^T x  (s-chunks of 128 on partitions, t on free)
    expT = exp(scoresT)                 (ACT + DVE Schraudolph, split for balance)
    o_un[c,t], denom[t] = [1;g]^T-weighted matmul over s   (PE, ones-FIRST row)
    recip = 1/denom  (row 0 of PSUM -> custom DVE at base partition 0)
    rb = partition_broadcast(recip)     (GPSIMD)
    o_norm = o_un * rb                  (DVE)
    y = (gamma*[0;Wo]) @ o_norm         (PE, k=33 with zero row 0)
    out = y + x                         (DVE, bf16 x residual)

v2 changes vs v1: x pre-replicated on host to [128, HW] (one bf16 tensor, no
SWDGE replica DMAs, no fp32 x), ones-first denominator (no lane-32 drain DMA),
all DMAs on HWDGE rings.
"""

import numpy as np
import ml_dtypes

import concourse.bass as bass
import concourse.bacc as bacc
import concourse.tile as tile
from concourse import mybir
from concourse.bass_utils import run_bass_kernel_spmd

FP32 = mybir.dt.float32
BF16 = mybir.dt.bfloat16
I16 = mybir.dt.int16
SCH_A = float(128.0 / np.log(2.0))   # Schraudolph bf16: round(A*x+B) -> bf16 bits
SCH_B = 16256.0 - 5.6
ts = bass.ts

C = 64
HW = 4096          # 64*64
S = 1024           # pooled spatial (32*32)
NSAMP = 2          # samples per core
NCHUNK = 8         # both t-chunks (512 wide) and conv chunks
TC = 512           # t-chunk width
SC = 128           # s-chunk width
N_DVE_EXP = 7      # of 32 exp quarter-units per sample, how many go to DVE


def build_nc(n_samples: int = NSAMP, repeat: int = 1) -> bass.Bass:
    nc = bacc.Bacc("TRN2", target_bir_lowering=False, debug=False)

    # x replicated on host: rows 0-63 == rows 64-127 == x (bf16)
    x2 = nc.dram_tensor("x2", [n_samples, 2 * C, HW], BF16, kind="ExternalInput").ap()
    # convA weights: columns 0-31 -> g channels (Wg), columns 32-39 -> phi (Wp)
    w_gp = nc.dram_tensor("w_gp", [C, 40], BF16, kind="ExternalInput").ap()
    # phi~ weights: rows 32-39 hold [Wt | Wt] [8, 128]; rows 0-31 zeros
    w_tt = nc.dram_tensor("w_tt", [40, 2 * C], BF16, kind="ExternalInput").ap()
    # final conv weights: row 0 zeros, rows 1-32 = (gamma*Wo)^T  [33, 64]
    w_o = nc.dram_tensor("w_o", [33, C], BF16, kind="ExternalInput").ap()
    ident = nc.dram_tensor("ident", [32, 32], BF16, kind="ExternalInput").ap()
    out = nc.dram_tensor("out", [n_samples, C, HW], FP32, kind="ExternalOutput").ap()

    with tile.TileContext(nc) as tc:
        for _ in range(repeat):
            _body(tc, n_samples, x2, w_gp, w_tt, w_o, ident, out)
    nc.compile()
    return nc


def _body(tc, n_samples, x2, w_gp, w_tt, w_o, ident, out):
    nc = tc.nc
    from contextlib import ExitStack

    with ExitStack() as ctx:
        consts = ctx.enter_context(tc.tile_pool(name="consts", bufs=1))
        xpool = ctx.enter_context(tc.tile_pool(name="xpool", bufs=2))
        mid = ctx.enter_context(tc.tile_pool(name="mid", bufs=2))
        expp = ctx.enter_context(tc.tile_pool(name="expp", bufs=9))
        smal = ctx.enter_context(tc.tile_pool(name="smal", bufs=6))
        outp = ctx.enter_context(tc.tile_pool(name="outp", bufs=2))
        ps_conv = ctx.enter_context(tc.tile_pool(name="ps_conv", bufs=1, space="PSUM"))
        ps_scorA = ctx.enter_context(tc.tile_pool(name="ps_scorA", bufs=1, space="PSUM"))
        ps_scorB = ctx.enter_context(tc.tile_pool(name="ps_scorB", bufs=1, space="PSUM"))
        ps_oacc = ctx.enter_context(tc.tile_pool(name="ps_oacc", bufs=2, space="PSUM"))
        ps_fin = ctx.enter_context(tc.tile_pool(name="ps_fin", bufs=1, space="PSUM"))

        # warm the ACT exp table set during setup (table load is ~2.7us)
        warm = consts.tile([1, 1], FP32)
        nc.vector.memset(warm[:], 0.0)
        nc.scalar.activation(warm[:], warm[:], mybir.ActivationFunctionType.Exp)

        wgp_sb = consts.tile([C, 40], BF16)
        wtt_sb = consts.tile([40, 2 * C], BF16)
        wo_sb = consts.tile([33, C], BF16)
        id_sb = consts.tile([32, 32], BF16)
        nc.scalar.dma_start(wgp_sb[:], w_gp[:])
        nc.scalar.dma_start(wtt_sb[:], w_tt[:])
        nc.scalar.dma_start(wo_sb[:], w_o[:])
        nc.scalar.dma_start(id_sb[:], ident[:])

        # ---- setup phase for every sample first (overlaps with attention of
        # earlier samples via scheduler priorities) ----------------------------
        setup = []
        for i in range(n_samples):
            xb = xpool.tile([2 * C, HW], BF16, tag="xb")
            pooled = mid.tile([40, S], BF16, tag="pooled")
            phi2 = mid.tile([2 * C, S], BF16, tag="phi2")
            gT = mid.tile([SC, 33 * NCHUNK], BF16, tag="gT")

            # x load: 4 DMAs of [128, 1024] on the SP HWDGE ring
            for c in range(4):
                nc.sync.dma_start(xb[:, ts(c, HW // 4)], x2[i][:, ts(c, HW // 4)])

            # convA (g + phi) + 2x2 maxpool, per 512-col chunk
            for c in range(NCHUNK):
                pa = ps_conv.tile([40, TC], FP32, tag="conv")
                nc.tensor.matmul(pa[:], wgp_sb[:], xb[0:C, ts(c, TC)])
                v = pa[:].rearrange("p (h eh w ew) -> p h w eh ew", h=4, eh=2, w=32, ew=2)
                pv = pooled[:, ts(c, SC)].rearrange("p (h w) -> p h w", h=4, w=32)
                nc.vector.tensor_reduce(
                    pv, v, axis=mybir.AxisListType.XY, op=mybir.AluOpType.max,
                    opt_input=False,
                )

            # phi~ for 512-wide s-chunks: [128, 512] (both replicas at once)
            for c in range(2):
                ppt = ps_fin.tile([2 * C, TC], FP32, tag="fin")
                nc.tensor.matmul(ppt[:], wtt_sb[32:40, :], pooled[32:40, ts(c, TC)])
                nc.vector.tensor_copy(phi2[:, ts(c, TC)], ppt[:])

            # g'^T chunks [128, 33] with ones column FIRST (denominator row 0)
            ones_view = gT[:].rearrange("p (k c) -> p k c", k=NCHUNK, c=33)
            nc.vector.memset(ones_view[:, :, 0:1], 1.0)
            for k in range(NCHUNK):
                pt = ps_conv.tile([SC, 32], BF16, tag="conv")
                nc.tensor.transpose(pt[:], pooled[0:32, ts(k, SC)], id_sb[:])
                nc.vector.tensor_copy(gT[:, 33 * k + 1 : 33 * k + 33], pt[:])
            setup.append((xb, phi2, gT))

        for i in range(n_samples):
            xb, phi2, gT = setup[i]
            # ---- attention main loop over t-chunks, software-pipelined:
            # emit scores+exp for chunk t, but o'+tail for chunk t-1, so a
            # po-slot stall never head-of-line-blocks the next scores on PE.
            o_norm = mid.tile([33, HW], BF16, tag="o_norm")
            out_sb = outp.tile([C, HW], FP32, tag="out_sb")
            pending = None  # (t, expT list)

            def emit_scores(t):
                exps = [None] * 4
                for q in range(4):
                    pool_q = ps_scorA if q % 2 == 0 else ps_scorB
                    pscr = pool_q.tile([SC, 2 * TC], FP32, tag="scor")
                    nc.tensor.matmul(
                        pscr[:, ts(0, TC)],
                        phi2[0:C, ts(2 * q, SC)],
                        xb[0:C, ts(t, TC)],
                        tile_position=(0, 0),
                    )
                    nc.tensor.matmul(
                        pscr[:, ts(1, TC)],
                        phi2[C : 2 * C, ts(2 * q + 1, SC)],
                        xb[C : 2 * C, ts(t, TC)],
                        tile_position=(64, 0),
                    )
                    # exp engine split: DVE Schraudolph for the first
                    # N_DVE_EXP (t, q) units, ACT exp for the rest
                    if 4 * t + q < N_DVE_EXP:
                        e16 = expp.tile([SC, 2 * TC], I16, tag="expT")
                        nc.vector.tensor_scalar(
                            e16[:], pscr[:], SCH_A, SCH_B,
                            mybir.AluOpType.mult, mybir.AluOpType.add,
                        )
                        exps[q] = e16[:].bitcast(BF16)
                    else:
                        et = expp.tile([SC, 2 * TC], BF16, tag="expT")
                        nc.scalar.activation(
                            et[:], pscr[:], mybir.ActivationFunctionType.Exp
                        )
                        exps[q] = et[:]
                return exps

            def emit_ovalue(t, exps):
                po = ps_oacc.tile([33, TC], FP32, tag="oacc")
                for q in range(4):
                    for j in range(2):
                        sc = 2 * q + j
                        nc.tensor.matmul(
                            po[:],
                            gT[:, 33 * sc : 33 * sc + 33],
                            exps[q][:, ts(j, TC)],
                            start=(sc == 0),
                            stop=(sc == 7),
                        )
                # denominator sits at partition 0 (ones-first): recip there,
                # broadcast to 33 partitions, normalize all 33 rows (row 0
                # becomes ~1, ignored by the zero row of w_o)
                rsb = smal.tile([1, TC], FP32, tag="rsb")
                nc.vector.reciprocal_approx_fast(rsb[:], po[0:1, :])
                rb = smal.tile([33, TC], FP32, tag="rb")
                nc.gpsimd.partition_broadcast(rb[:], rsb[:])
                nc.vector.tensor_mul(o_norm[:, ts(t, TC)], po[:], rb[:])
                py = ps_fin.tile([C, TC], FP32, tag="fin")
                nc.tensor.matmul(py[:], wo_sb[:], o_norm[:, ts(t, TC)])
                nc.vector.tensor_add(out_sb[:, ts(t, TC)], py[:], xb[0:C, ts(t, TC)])
                nc.sync.dma_start(out[i][:, ts(t, TC)], out_sb[:, ts(t, TC)])

            for t in range(NCHUNK):
                exps = emit_scores(t)
                if pending is not None:
                    emit_ovalue(*pending)
                if t == NCHUNK - 1:
                    emit_ovalue(t, exps)
                    pending = None
                else:
                    pending = (t, exps)


# ---------------------------------------------------------------------------
# host-side driver
# ---------------------------------------------------------------------------

def _prep_consts(Wt, Wp, Wg, Wo, gamma):
    bf = ml_dtypes.bfloat16
    w_gp = np.zeros((C, 40), np.float32)
    w_gp[:, 0:32] = Wg.T
    w_gp[:, 32:40] = Wp.T
    w_tt = np.zeros((40, 2 * C), np.float32)
    w_tt[32:40, 0:C] = Wt
    w_tt[32:40, C : 2 * C] = Wt
    w_o = np.zeros((33, C), np.float32)
    w_o[1:33, :] = (np.float32(gamma) * np.asarray(Wo, np.float32)).T
    ident = np.eye(32, dtype=np.float32)
    return {
        "w_gp": w_gp.astype(bf),
        "w_tt": w_tt.astype(bf),
        "w_o": np.ascontiguousarray(w_o).astype(bf),
        "ident": ident.astype(bf),
    }


def make_in_maps(x, Wt, Wp, Wg, Wo, gamma, n_cores=8):
    bf = ml_dtypes.bfloat16
    x = np.asarray(x, dtype=np.float32)
    B = x.shape[0]
    nper = B // n_cores
    xr = np.ascontiguousarray(x.reshape(B, C, HW)).astype(bf)
    x2 = np.concatenate([xr, xr], axis=1)  # [B, 128, HW] host-replicated
    consts = _prep_consts(
        np.asarray(Wt, np.float32),
        np.asarray(Wp, np.float32),
        np.asarray(Wg, np.float32),
        np.asarray(Wo, np.float32),
        np.float32(gamma),
    )
    in_maps = []
    for cid in range(n_cores):
        in_maps.append({"x2": x2[cid * nper : (cid + 1) * nper], **consts})
    return in_maps


def kernel(x, Wt, Wp, Wg, Wo, gamma):
    x = np.asarray(x, dtype=np.float32)
    B = x.shape[0]
    n_cores = 8
    nper = B // n_cores
    in_maps = make_in_maps(x, Wt, Wp, Wg, Wo, gamma, n_cores)
    nc = build_nc(nper)
    res = run_bass_kernel_spmd(nc, in_maps, core_ids=list(range(n_cores)))
    outs = [res.results[cid]["out"] for cid in range(n_cores)]
    return np.concatenate(outs, axis=0).reshape(B, C, 64, 64)
